# revision 39
# baseline (speedup 1.0000x reference)
"""Mamba SSM block on 8 TRN2 NeuronCores (Bass/Tile, SPMD).

Sharding: d_inner (2048 -> 256/core) for in_proj, conv, dt/B/C projections and
the selective scan; output projection is token-sharded (512 tokens/core).
Collectives per run:
  - AllReduce of x_dbl projection partials [96, 512] bf16 per 512-token chunk
  - Three AllToAlls of the gated scan output yg (chunks 0-3 / 4-5 / 6-7) that
    reshard yg from d_inner-sharded to token-sharded; each core then computes
    the full (host-folded) W_c = W_out @ W_out_ssm projection for its tokens.

Scan: h[t] = exp(A dt[t]) h[t-1] + (dt[t] x[t]) B[t] via 16 tensor_tensor_scan
ops (one per state index) along tokens, chained across chunks through
per-partition `initial` APs. B/C are broadcast to 128 partitions by a single
stride-0-partition DMA per chunk (no PE/ACT involvement). silu/softplus run
natively on the ACT engine. Matmul operands and scan tensors are bf16
(rel_err ~5e-3 validated vs fp32 reference); conv, dt and the AllReduce stay
fp32.
"""
import numpy as np
import ml_dtypes

import concourse.bass as bass
import concourse.tile as tile
from concourse import bacc, mybir
from concourse.bass_utils import run_bass_kernel_spmd

BFnp = ml_dtypes.bfloat16
F32 = mybir.dt.float32
BF16 = mybir.dt.bfloat16
AF = mybir.ActivationFunctionType
OP = mybir.AluOpType

NC = 8
B, L, DM = 2, 2048, 1024
DI, S, R, KC = 2048, 16, 64, 4
DIL = DI // NC            # 256 d_inner per core
NT = B * L                # 4096 tokens (batch-major)
TC = 512                  # tokens per chunk
NCH = NT // TC            # 8 chunks
NI = DIL // 128           # 2 partition tiles of local d_inner
EO = 8                    # 1024 output rows = 8 tiles of 128

# AllToAll groups: (token_start, token_count, tokens-per-block); the last
# chunk is processed as two half-width mids so the final (fully exposed)
# AllToAll + projection covers only 256 tokens
A2A_GROUPS = [(0, 2048, 256), (2048, 1024, 128), (3072, 768, 96), (3840, 256, 32)]

_NC_CACHE = {}


def _narrow_act_tables(arch):
    """Keep Exp and Ln only in natural_log_exp_and_others so the compiler's
    table-load pass puts the whole dt/a_t sequence on one activation table
    (it otherwise thrashes between exp_and_others and natural_log, ~1.3us
    per reload). Only removes entries, so every emitted load is still valid."""
    from concourse.hw_specs import get_activation_tables
    tabs = get_activation_tables(arch)  # functools.cache -> shared dict
    for name, fns in tabs.items():
        if name == "natural_log_exp_and_others":
            continue
        fns.discard(AF.Exp)
        fns.discard(AF.Ln)


def build(use_silu=True):
    """use_silu=False replaces native Silu with Sigmoid+mult (CoreSim lacks Silu)."""
    key = ("nc", use_silu)
    if key in _NC_CACHE:
        return _NC_CACHE[key]
    nc = bacc.Bacc("TRN2", target_bir_lowering=False, debug=False, num_devices=NC)
    _NC_CACHE[key] = None  # placeholder
    _narrow_act_tables(nc.m.arch)

    # ---- per-core DRAM inputs (host pre-sharded / transposed / casted) ----
    x_t = nc.dram_tensor("x_t", [DM, NT], BF16, kind="ExternalInput")         # replicated
    w_in_x = nc.dram_tensor("w_in_x", [DM, DIL], BF16, kind="ExternalInput")  # W_in[dk,:].T
    w_in_z = nc.dram_tensor("w_in_z", [DM, DIL], BF16, kind="ExternalInput")
    conv_wd = nc.dram_tensor("conv_wd", [128, NI * KC * 128], BF16, kind="ExternalInput")  # diag tap blocks
    conv_b = nc.dram_tensor("conv_b", [DIL, 1], F32, kind="ExternalInput")
    w_xp = nc.dram_tensor("w_xp", [DIL, R + 2 * S], BF16, kind="ExternalInput")  # W_xp[:,dk].T
    w_dt = nc.dram_tensor("w_dt", [R, DIL], BF16, kind="ExternalInput")          # W_dt[dk,:].T
    b_dt = nc.dram_tensor("b_dt", [DIL, 1], F32, kind="ExternalInput")
    a_mat = nc.dram_tensor("a_mat", [DIL, S], F32, kind="ExternalInput")        # -exp(A_log[dk])
    d_vec = nc.dram_tensor("d_vec", [DIL, 1], F32, kind="ExternalInput")
    w_c = nc.dram_tensor("w_c", [128, 16 * EO * 128], BF16, kind="ExternalInput")  # W_c blocks, replicated
    b_o = nc.dram_tensor("b_o", [128, EO], F32, kind="ExternalInput")
    out = nc.dram_tensor("out", [8 * 128, TC], F32, kind="ExternalOutput")     # [1024, my 512 tokens]

    with tile.TileContext(nc) as tc:
        with (
            tc.tile_pool(name="wpool", bufs=1) as wp,     # persistent weights
            tc.tile_pool(name="xckp", bufs=1) as xkp,     # streamed x
            tc.tile_pool(name="ygsp", bufs=1) as ygp,     # A2A output gather
            tc.tile_pool(name="work", bufs=1) as wk,      # DVE-only transients
            tc.tile_pool(name="worka", bufs=2) as wka,    # ACT/DMA-written tiles
            tc.tile_pool(name="keep", bufs=6) as kp,      # xs/g (live across chunk)
            tc.tile_pool(name="scan", bufs=1) as sc,      # big bf16 scan tiles
            tc.tile_pool(name="scana", bufs=2) as sca,    # a_t (dbl buf: ACT vs scan)
            tc.tile_pool(name="bcast", bufs=1) as bcp,    # B/C broadcast tiles
            tc.tile_pool(name="state", bufs=1) as st,     # persistent hprev/xtail
            tc.tile_pool(name="psA", bufs=2, space="PSUM") as psA,
            tc.tile_pool(name="psC", bufs=1, space="PSUM") as psC,
            tc.tile_pool(name="psB1", bufs=1, space="PSUM") as psB1,
            tc.tile_pool(name="psB2", bufs=1, space="PSUM") as psB2,
            tc.tile_pool(name="psO", bufs=1, space="PSUM") as psO,
            tc.tile_pool(name="dram", bufs=1, space="DRAM") as dr,
        ):
            # ---------- load weights ----------
            winx = wp.tile([128, 8 * NI * 128], BF16, tag="winx")
            winz = wp.tile([128, 8 * NI * 128], BF16, tag="winz")
            nc.sync.dma_start(
                winx[:].rearrange("p (k i n) -> p k i n", k=8, i=NI),
                w_in_x.ap().rearrange("(k p) (i n) -> p k i n", p=128, i=NI))
            nc.sync.dma_start(
                winz[:].rearrange("p (k i n) -> p k i n", k=8, i=NI),
                w_in_z.ap().rearrange("(k p) (i n) -> p k i n", p=128, i=NI))
            wxp = wp.tile([128, NI * (R + 2 * S)], BF16, tag="wxp")
            nc.sync.dma_start(
                wxp[:].rearrange("p (i n) -> p i n", i=NI),
                w_xp.ap().rearrange("(i p) n -> p i n", p=128))
            wdt = wp.tile([R, NI * 128], BF16, tag="wdt")
            nc.sync.dma_start(wdt[:], w_dt[:, :])
            cwd = wp.tile([128, NI * KC * 128], BF16, tag="cwd")
            nc.sync.dma_start(cwd[:], conv_wd[:, :])
            cb = wp.tile([128, NI], F32, tag="cb")
            bdt = wp.tile([128, NI], F32, tag="bdt")
            dv = wp.tile([128, NI], F32, tag="dv")
            am = wp.tile([128, NI * S], F32, tag="am")
            for i in range(NI):
                sl = slice(i * 128, (i + 1) * 128)
                nc.sync.dma_start(cb[:, i:i + 1], conv_b[sl, :])
                nc.sync.dma_start(bdt[:, i:i + 1], b_dt[sl, :])
                nc.sync.dma_start(dv[:, i:i + 1], d_vec[sl, :])
                nc.sync.dma_start(am[:, i * S:(i + 1) * S], a_mat[sl, :])
            bo = wp.tile([128, EO], F32, tag="bo")
            nc.sync.dma_start(bo[:], b_o[:, :])

            hprev = st.tile([128, NI * S], BF16, tag="hprev")
            xtail = st.tile([128, NI * 3], BF16, tag="xtail")

            # A2A input/output DRAM tiles (local, not Shared)
            a2a_in = [dr.tile([2048, tb], BF16, tag=f"a2ain{g}", name=f"a2ain{g}")
                      for g, (_, _, tb) in enumerate(A2A_GROUPS)]
            a2a_out = [dr.tile([2048, tb], BF16, tag=f"a2aout{g}", name=f"a2aout{g}")
                       for g, (_, _, tb) in enumerate(A2A_GROUPS)]

            ctx = {}

            def front(c):
                """x stream, in_proj, conv, silu, z-gate, x_dbl partial, AR issue."""
                t0 = c * TC
                reset = (c % (NCH // B) == 0)

                xck = xkp.tile([128, 8 * TC], BF16, tag="xck")
                nc.sync.dma_start(
                    xck[:].rearrange("p (k t) -> p k t", k=8),
                    x_t.ap()[:, t0:t0 + TC].rearrange("(k p) t -> p k t", p=128))

                xs_i, g_i = [], []
                for i in range(NI):
                    ps_x = psA.tile([128, TC], F32, tag="psx")
                    ps_z = psA.tile([128, TC], F32, tag="psz")
                    for kt in range(8):
                        wsl = slice((kt * NI + i) * 128, (kt * NI + i + 1) * 128)
                        nc.tensor.matmul(ps_x[:], winx[:, wsl], xck[:, kt * TC:(kt + 1) * TC],
                                         start=(kt == 0), stop=(kt == 7))
                    for kt in range(8):
                        wsl = slice((kt * NI + i) * 128, (kt * NI + i + 1) * 128)
                        nc.tensor.matmul(ps_z[:], winz[:, wsl], xck[:, kt * TC:(kt + 1) * TC],
                                         start=(kt == 0), stop=(kt == 7))

                    # causal depthwise conv as 4 accumulating diag matmuls on PE;
                    # xin carries a 3-token halo from the previous chunk
                    xin = wka.tile([128, TC + 3], BF16, tag="xin")
                    nc.scalar.copy(xin[:, 3:], ps_x[:])
                    if reset:
                        nc.gpsimd.memset(xin[:, 0:3], 0.0)
                    else:
                        nc.vector.tensor_copy(xin[:, 0:3], xtail[:, i * 3:i * 3 + 3])
                    nc.vector.tensor_copy(xtail[:, i * 3:i * 3 + 3], ps_x[:, TC - 3:TC])
                    ps_c = psC.tile([128, TC], F32, tag="psc")
                    for k in range(KC):
                        nc.tensor.matmul(ps_c[:], cwd[:, (i * KC + k) * 128:(i * KC + k + 1) * 128],
                                         xin[:, k:k + TC], start=(k == 0), stop=(k == KC - 1))
                    xs = kp.tile([128, TC], BF16, tag="xs")
                    g = kp.tile([128, TC], BF16, tag="g")
                    if use_silu:
                        nc.scalar.activation(xs[:], ps_c[:], AF.Silu, bias=cb[:, i:i + 1])
                        # z gate: g = silu(z) straight from PSUM
                        nc.scalar.activation(g[:], ps_z[:], AF.Silu)
                    else:
                        u = wk.tile([128, TC], F32, tag="accA")
                        nc.scalar.activation(u[:], ps_c[:], AF.Identity, bias=cb[:, i:i + 1])
                        sgu = wka.tile([128, TC], F32, tag="sgu")
                        nc.scalar.activation(sgu[:], u[:], AF.Sigmoid)
                        nc.vector.tensor_tensor(out=xs[:], in0=u[:], in1=sgu[:], op=OP.mult)
                        sgz = wka.tile([128, TC], F32, tag="sgz")
                        nc.scalar.activation(sgz[:], ps_z[:], AF.Sigmoid)
                        nc.vector.scalar_tensor_tensor(
                            out=g[:], in0=sgz[:], scalar=1.0, in1=ps_z[:],
                            op0=OP.mult, op1=OP.mult)
                    xs_i.append(xs)
                    g_i.append(g)

                # x_dbl partial + AllReduce
                ps_xd = psB1.tile([R + 2 * S, TC], F32, tag="psxd")
                for i in range(NI):
                    nc.tensor.matmul(ps_xd[:], wxp[:, i * (R + 2 * S):(i + 1) * (R + 2 * S)],
                                     xs_i[i][:], start=(i == 0), stop=(i == NI - 1))
                xd_sb = wka.tile([R + 2 * S, TC], BF16, tag="xdsb")
                nc.scalar.copy(xd_sb[:], ps_xd[:])
                xd_part = dr.tile([R + 2 * S, TC], BF16, tag=f"xdp{c % 4}")
                nc.sync.dma_start(xd_part[:], xd_sb[:])
                xd_red = nc.dram_tensor(f"xd_red_{c}", [R + 2 * S, TC], BF16, addr_space="Shared")
                nc.gpsimd.collective_compute(
                    "AllReduce", OP.add, replica_groups=[list(range(NC))],
                    ins=[xd_part[:]], outs=[xd_red.ap()])
                ctx[c] = dict(xs_i=xs_i, g_i=g_i, xd_red=xd_red)

            def mid_head(c):
                """dtr/B/C loads and dt for both i-tiles (full chunk width)."""
                xd_red = ctx[c]["xd_red"]
                dtr = wka.tile([R, TC], BF16, tag="dtr")
                nc.sync.dma_start(dtr[:], xd_red.ap()[0:R, :])
                # B/C rows broadcast to all 128 partitions: one stride-0 DMA each
                b_bc = bcp.tile([128, S * TC], BF16, tag="bbc")
                c_bc = bcp.tile([128, S * TC], BF16, tag="cbc")
                nc.scalar.dma_start(
                    b_bc[:].rearrange("p (s t) -> p s t", s=S),
                    xd_red.ap()[R:R + S, :].unsqueeze(0).broadcast_to([128, S, TC]))
                nc.scalar.dma_start(
                    c_bc[:].rearrange("p (s t) -> p s t", s=S),
                    xd_red.ap()[R + S:R + 2 * S, :].unsqueeze(0).broadcast_to([128, S, TC]))
                # Exp+Ln softplus keeps the whole mid phase on the
                # natural_log_exp activation table (a_t uses Exp too)
                dt_i = []
                for i in range(NI):
                    ps_dt = psB2.tile([128, TC], F32, tag="psdt")
                    nc.tensor.matmul(ps_dt[:], wdt[:, i * 128:(i + 1) * 128], dtr[:],
                                     start=True, stop=True)
                    edt = wka.tile([128, TC], F32, tag="edt")
                    nc.scalar.activation(edt[:], ps_dt[:], AF.Exp, bias=bdt[:, i:i + 1])
                    dt = wka.tile([128, TC], BF16, tag="dtt")
                    nc.scalar.activation(dt[:], edt[:], AF.Ln, bias=1.0)
                    dt_i.append(dt)
                ctx[c].update(b_bc=b_bc, c_bc=c_bc, dt_i=dt_i)

            def mid_body(c, col0, W):
                """dtx, decay exps, bb, scans, h*C + folds, gate, yg scatter for
                token window [c*TC+col0, c*TC+col0+W)."""
                t0 = c * TC + col0
                reset = (t0 % (NT // B) == 0)
                xs_i, g_i = ctx[c]["xs_i"], ctx[c]["g_i"]
                b_bc, c_bc, dt_i = ctx[c]["b_bc"], ctx[c]["c_bc"], ctx[c]["dt_i"]
                win = slice(col0, col0 + W)

                for g, (gt0, gnt, tb) in enumerate(A2A_GROUPS):
                    if gt0 <= t0 < gt0 + gnt:
                        grp, tb_g, rel = g, tb, t0 - gt0
                        break

                # phase A per i-tile: dtx, decay exps, bb, scans, h*C, folds.
                r2_i = []
                for i in range(NI):
                    dt = dt_i[i]
                    dtx = wk.tile([128, TC], BF16, tag="dtx")
                    nc.vector.tensor_tensor(out=dtx[:, 0:W], in0=dt[:, win],
                                            in1=xs_i[i][:, win], op=OP.mult)

                    a_t = sca.tile([128, S * TC], BF16, tag="a_t")
                    for s in range(S):
                        nc.scalar.activation(
                            a_t[:, s * TC + col0:s * TC + col0 + W], dt[:, win], AF.Exp,
                            scale=am[:, i * S + s:i * S + s + 1])

                    bb_t = sc.tile([128, S * TC], BF16, tag="bb_t")
                    bb3 = bb_t[:].rearrange("p (s t) -> p s t", s=S)
                    nc.vector.tensor_tensor(
                        out=bb3[:, :, win],
                        in0=dtx[:, 0:W].unsqueeze(1).broadcast_to([128, S, W]),
                        in1=b_bc[:].rearrange("p (s t) -> p s t", s=S)[:, :, win],
                        op=OP.mult)

                    h_t = sc.tile([128, S * TC], BF16, tag="h_t")
                    if reset:
                        nc.gpsimd.memset(hprev[:, i * S:(i + 1) * S], 0.0)
                    for s in range(S):
                        nc.vector.tensor_tensor_scan(
                            h_t[:, s * TC + col0:s * TC + col0 + W],
                            a_t[:, s * TC + col0:s * TC + col0 + W],
                            bb_t[:, s * TC + col0:s * TC + col0 + W],
                            hprev[:, i * S + s:i * S + s + 1],
                            op0=OP.mult, op1=OP.add)
                    nc.vector.tensor_copy(
                        hprev[:, i * S:(i + 1) * S],
                        h_t[:].rearrange("p (s t) -> p s t", s=S)[:, :, col0 + W - 1])

                    hc_t = sc.tile([128, S * TC], BF16, tag="hc_t", bufs=2)
                    hc3 = hc_t[:].rearrange("p (s t) -> p s t", s=S)
                    nc.vector.tensor_tensor(
                        out=hc3[:, :, win],
                        in0=h_t[:].rearrange("p (s t) -> p s t", s=S)[:, :, win],
                        in1=c_bc[:].rearrange("p (s t) -> p s t", s=S)[:, :, win],
                        op=OP.mult)
                    # folds stay on DVE: serialized but uncontended beats
                    # cross-engine overlap that halves every op's SBUF rate
                    nc.vector.tensor_tensor(out=hc3[:, 0:8, win], in0=hc3[:, 0:8, win],
                                            in1=hc3[:, 8:16, win], op=OP.add)
                    nc.vector.tensor_tensor(out=hc3[:, 0:4, win], in0=hc3[:, 0:4, win],
                                            in1=hc3[:, 4:8, win], op=OP.add)
                    r2_i.append(hc3)

                for i in range(NI):
                    hc3 = r2_i[i]
                    r3 = wk.tile([128, 2 * TC], BF16, tag="r3")
                    nc.vector.tensor_tensor(
                        out=r3[:].rearrange("p (u t) -> p u t", u=2)[:, :, 0:W],
                        in0=hc3[:, 0:2, win], in1=hc3[:, 2:4, win], op=OP.add)
                    y = wk.tile([128, TC], F32, tag="y")
                    nc.vector.tensor_tensor(out=y[:, 0:W], in0=r3[:, 0:W],
                                            in1=r3[:, TC:TC + W], op=OP.add)
                    yD = wk.tile([128, TC], F32, tag="yD")
                    nc.vector.scalar_tensor_tensor(
                        out=yD[:, 0:W], in0=xs_i[i][:, win], scalar=dv[:, i:i + 1],
                        in1=y[:, 0:W], op0=OP.mult, op1=OP.add)
                    yg = wk.tile([128, TC], BF16, tag="yg")
                    nc.vector.tensor_tensor(out=yg[:, 0:W], in0=yD[:, 0:W],
                                            in1=g_i[i][:, win], op=OP.mult)
                    # scatter yg into the A2A input: row-block = token-block,
                    # rows within block = my local d channels
                    for b in range(rel // tb_g, (rel + W + tb_g - 1) // tb_g):
                        colA = max(b * tb_g - rel, 0)
                        colB = min((b + 1) * tb_g - rel, W)
                        nc.sync.dma_start(
                            a2a_in[grp][b * 256 + i * 128: b * 256 + (i + 1) * 128,
                                        rel + colA - b * tb_g: rel + colB - b * tb_g],
                            yg[:, colA:colB])

            def a2a(g):
                nc.gpsimd.collective_compute(
                    "AllToAll", OP.bypass, replica_groups=[list(range(NC))],
                    ins=[a2a_in[g][:]], outs=[a2a_out[g][:]])

            def tail(g):
                """gather a2a_out[g], out projection, bias, store my token cols."""
                tb = A2A_GROUPS[g][2]
                col0 = sum(t for _, _, t in A2A_GROUPS[:g])
                ygs = ygp.tile([128, 16 * tb], BF16, tag="ygs")
                nc.sync.dma_start(
                    ygs[:].rearrange("p (k t) -> p k t", k=16),
                    a2a_out[g][:, :].rearrange("(k p) t -> p k t", p=128))
                for e in range(EO):
                    wce = ygp.tile([128, 16 * 128], BF16, tag="wce", bufs=2)
                    nc.sync.dma_start(wce[:], w_c.ap()[:, e * 2048:(e + 1) * 2048])
                    ps_o = psO.tile([128, tb], F32, tag="pso")
                    for kt in range(16):
                        nc.tensor.matmul(ps_o[:], wce[:, kt * 128:(kt + 1) * 128],
                                         ygs[:, kt * tb:(kt + 1) * tb],
                                         start=(kt == 0), stop=(kt == 15))
                    o_sb = wka.tile([128, tb], F32, tag="osb")
                    nc.scalar.activation(o_sb[:], ps_o[:], AF.Identity, bias=bo[:, e:e + 1])
                    nc.sync.dma_start(out[e * 128:(e + 1) * 128, col0:col0 + tb], o_sb[:])

            for c in range(NCH + 3):
                if c < NCH:
                    front(c)
                m = c - 2
                if 0 <= m < 7:
                    mid_head(m)
                    mid_body(m, 0, TC)
                    if m == 3:
                        a2a(0)
                    elif m == 5:
                        a2a(1)
                elif m == 7:
                    mid_head(7)
                    mid_body(7, 0, 256)
                    a2a(2)
                    tail(2)
                    mid_body(7, 256, 256)
                    a2a(3)
                if c == 7:
                    tail(0)
                elif c == 8:
                    tail(1)
                elif c == 10:
                    tail(3)

    nc.compile()
    _NC_CACHE[key] = nc
    return nc


def _prep_inputs(inputs):
    x = np.ascontiguousarray(np.asarray(inputs["x"], np.float32))
    W_in = np.asarray(inputs["W_in"], np.float32)
    conv_w = np.asarray(inputs["conv_w"], np.float32)
    conv_b = np.asarray(inputs["conv_b"], np.float32)
    W_xp = np.asarray(inputs["W_xp"], np.float32)
    W_dt = np.asarray(inputs["W_dt"], np.float32)
    b_dt = np.asarray(inputs["b_dt"], np.float32)
    A_log = np.asarray(inputs["A_log"], np.float32)
    D = np.asarray(inputs["D"], np.float32)
    W_out_ssm = np.asarray(inputs["W_out_ssm"], np.float32)
    W_out = np.asarray(inputs["W_out"], np.float32)
    b_out = np.asarray(inputs["b_out"], np.float32)

    A = -np.exp(A_log)
    W_c = (W_out.astype(np.float64) @ W_out_ssm.astype(np.float64)).astype(np.float32)
    x_t = np.ascontiguousarray(x.reshape(NT, DM).T.astype(BFnp))  # [DM, NT] bf16

    # conv taps as diagonal stationary blocks per (local i-tile, tap)
    def conv_diag(dsl):
        cw = conv_w[dsl, 0, :]  # [DIL, KC]
        blk = np.zeros((128, NI * KC * 128), BFnp)
        for i in range(NI):
            for k in range(KC):
                d = np.zeros((128, 128), np.float32)
                np.fill_diagonal(d, cw[i * 128:(i + 1) * 128, k])
                blk[:, (i * KC + k) * 128:(i * KC + k + 1) * 128] = d.astype(BFnp)
        return blk

    # W_c as stationary blocks: [128, (kt*EO+e)*128] = W_c[e-rows, kt-cols].T
    wc_blk = np.zeros((128, 16 * EO * 128), BFnp)
    for e in range(EO):
        for kt in range(16):
            blk = W_c[e * 128:(e + 1) * 128, kt * 128:(kt + 1) * 128].T
            wc_blk[:, (e * 16 + kt) * 128:(e * 16 + kt + 1) * 128] = blk.astype(BFnp)
    bo_blk = np.ascontiguousarray(b_out.reshape(EO, 128).T)  # [128, EO]

    in_maps = []
    for k in range(NC):
        dsl = slice(k * DIL, (k + 1) * DIL)
        in_maps.append({
            "x_t": x_t,
            "w_in_x": np.ascontiguousarray(W_in[dsl, :].T.astype(BFnp)),
            "w_in_z": np.ascontiguousarray(
                W_in[DI + k * DIL: DI + (k + 1) * DIL, :].T.astype(BFnp)),
            "conv_wd": conv_diag(dsl),
            "conv_b": np.ascontiguousarray(conv_b[dsl][:, None]),
            "w_xp": np.ascontiguousarray(W_xp[:, dsl].T.astype(BFnp)),
            "w_dt": np.ascontiguousarray(W_dt[dsl, :].T.astype(BFnp)),
            "b_dt": np.ascontiguousarray(b_dt[dsl][:, None]),
            "a_mat": np.ascontiguousarray(A[dsl, :]),
            "d_vec": np.ascontiguousarray(D[dsl][:, None]),
            "w_c": wc_blk,
            "b_o": bo_blk,
        })
    return in_maps


def _assemble(results):
    full = np.zeros((DM, NT), np.float32)
    for k in range(NC):
        o = results[k]["out"]  # [1024, 512]: token cols per A2A group
        col = 0
        for tok0, ntok, tb in A2A_GROUPS:
            full[:, tok0 + tb * k: tok0 + tb * (k + 1)] = o[:, col:col + tb]
            col += tb
    return np.ascontiguousarray(full.T).reshape(B, L, DM)


def kernel(**inputs):
    nc = build()
    in_maps = _prep_inputs(inputs)
    res = run_bass_kernel_spmd(nc, in_maps, core_ids=list(range(NC)))
    return _assemble(res.results)


def kernel_sim(**inputs):
    """Run through MultiCoreSim instead of HW (for debugging)."""
    from concourse.bass_interp import MultiCoreSim
    nc = build(use_silu=False)
    in_maps = _prep_inputs(inputs)
    sim = MultiCoreSim(nc, num_cores=NC)
    for k in range(NC):
        for name, arr in in_maps[k].items():
            sim.cores[k].tensor(name)[:] = arr
    sim.simulate(check_with_hw=False)
    results = [{"out": sim.cores[k].tensor("out").copy()} for k in range(NC)]
    return _assemble(results)


# revision 41
# speedup vs baseline: 1.0045x; 1.0045x over previous
"""Mamba SSM block on 8 TRN2 NeuronCores (Bass/Tile, SPMD).

Sharding: d_inner (2048 -> 256/core) for in_proj, conv, dt/B/C projections and
the selective scan; output projection is token-sharded (512 tokens/core).
Collectives per run:
  - AllReduce of x_dbl projection partials [96, 512] bf16 per 512-token chunk
  - Three AllToAlls of the gated scan output yg (chunks 0-3 / 4-5 / 6-7) that
    reshard yg from d_inner-sharded to token-sharded; each core then computes
    the full (host-folded) W_c = W_out @ W_out_ssm projection for its tokens.

Scan: h[t] = exp(A dt[t]) h[t-1] + (dt[t] x[t]) B[t] via 16 tensor_tensor_scan
ops (one per state index) along tokens, chained across chunks through
per-partition `initial` APs. B/C are broadcast to 128 partitions by a single
stride-0-partition DMA per chunk (no PE/ACT involvement). silu/softplus run
natively on the ACT engine. Matmul operands and scan tensors are bf16
(rel_err ~5e-3 validated vs fp32 reference); conv, dt and the AllReduce stay
fp32.
"""
import numpy as np
import ml_dtypes

import concourse.bass as bass
import concourse.tile as tile
from concourse import bacc, mybir
from concourse.bass_utils import run_bass_kernel_spmd

BFnp = ml_dtypes.bfloat16
F32 = mybir.dt.float32
BF16 = mybir.dt.bfloat16
AF = mybir.ActivationFunctionType
OP = mybir.AluOpType

NC = 8
B, L, DM = 2, 2048, 1024
DI, S, R, KC = 2048, 16, 64, 4
DIL = DI // NC            # 256 d_inner per core
NT = B * L                # 4096 tokens (batch-major)
TC = 512                  # tokens per chunk
NCH = NT // TC            # 8 chunks
NI = DIL // 128           # 2 partition tiles of local d_inner
EO = 8                    # 1024 output rows = 8 tiles of 128

# AllToAll groups: (token_start, token_count, tokens-per-block); the last
# chunk is processed as two half-width mids so the final (fully exposed)
# AllToAll + projection covers only 256 tokens
A2A_GROUPS = [(0, 2048, 256), (2048, 1024, 128), (3072, 768, 96), (3840, 256, 32)]

_NC_CACHE = {}


def _narrow_act_tables(arch):
    """Keep Exp and Ln only in natural_log_exp_and_others so the compiler's
    table-load pass puts the whole dt/a_t sequence on one activation table
    (it otherwise thrashes between exp_and_others and natural_log, ~1.3us
    per reload). Only removes entries, so every emitted load is still valid."""
    from concourse.hw_specs import get_activation_tables
    tabs = get_activation_tables(arch)  # functools.cache -> shared dict
    for name, fns in tabs.items():
        if name == "natural_log_exp_and_others":
            continue
        fns.discard(AF.Exp)
        fns.discard(AF.Ln)


def build(use_silu=True):
    """use_silu=False replaces native Silu with Sigmoid+mult (CoreSim lacks Silu)."""
    key = ("nc", use_silu)
    if key in _NC_CACHE:
        return _NC_CACHE[key]
    nc = bacc.Bacc("TRN2", target_bir_lowering=False, debug=False, num_devices=NC)
    _NC_CACHE[key] = None  # placeholder
    _narrow_act_tables(nc.m.arch)

    # ---- per-core DRAM inputs (host pre-sharded / transposed / casted) ----
    x_t = nc.dram_tensor("x_t", [DM, NT], BF16, kind="ExternalInput")         # replicated
    w_in_x = nc.dram_tensor("w_in_x", [DM, DIL], BF16, kind="ExternalInput")  # W_in[dk,:].T
    w_in_z = nc.dram_tensor("w_in_z", [DM, DIL], BF16, kind="ExternalInput")
    conv_wd = nc.dram_tensor("conv_wd", [128, NI * KC * 128], BF16, kind="ExternalInput")  # diag tap blocks
    conv_b = nc.dram_tensor("conv_b", [DIL, 1], F32, kind="ExternalInput")
    w_xp = nc.dram_tensor("w_xp", [DIL, R + 2 * S], BF16, kind="ExternalInput")  # W_xp[:,dk].T
    w_dt = nc.dram_tensor("w_dt", [R, DIL], BF16, kind="ExternalInput")          # W_dt[dk,:].T
    b_dt = nc.dram_tensor("b_dt", [DIL, 1], F32, kind="ExternalInput")
    a_mat = nc.dram_tensor("a_mat", [DIL, S], F32, kind="ExternalInput")        # -exp(A_log[dk])
    d_vec = nc.dram_tensor("d_vec", [DIL, 1], F32, kind="ExternalInput")
    w_c = nc.dram_tensor("w_c", [128, 16 * EO * 128], BF16, kind="ExternalInput")  # W_c blocks, replicated
    b_o = nc.dram_tensor("b_o", [128, EO], F32, kind="ExternalInput")
    out = nc.dram_tensor("out", [8 * 128, TC], F32, kind="ExternalOutput")     # [1024, my 512 tokens]

    with tile.TileContext(nc) as tc:
        with (
            tc.tile_pool(name="wpool", bufs=1) as wp,     # persistent weights
            tc.tile_pool(name="xckp", bufs=2) as xkp,     # streamed x
            tc.tile_pool(name="ygsp", bufs=1) as ygp,     # A2A output gather
            tc.tile_pool(name="work", bufs=1) as wk,      # DVE-only transients
            tc.tile_pool(name="worka", bufs=2) as wka,    # ACT/DMA-written tiles
            tc.tile_pool(name="keep", bufs=6) as kp,      # xs/g (live across chunk)
            tc.tile_pool(name="scan", bufs=1) as sc,      # big bf16 scan tiles
            tc.tile_pool(name="scana", bufs=2) as sca,    # a_t (dbl buf: ACT vs scan)
            tc.tile_pool(name="bcast", bufs=1) as bcp,    # B/C broadcast tiles
            tc.tile_pool(name="state", bufs=1) as st,     # persistent hprev/xtail
            tc.tile_pool(name="psA", bufs=2, space="PSUM") as psA,
            tc.tile_pool(name="psC", bufs=1, space="PSUM") as psC,
            tc.tile_pool(name="psB1", bufs=1, space="PSUM") as psB1,
            tc.tile_pool(name="psB2", bufs=1, space="PSUM") as psB2,
            tc.tile_pool(name="psO", bufs=1, space="PSUM") as psO,
            tc.tile_pool(name="dram", bufs=1, space="DRAM") as dr,
        ):
            # ---------- load weights ----------
            winx = wp.tile([128, 8 * NI * 128], BF16, tag="winx")
            winz = wp.tile([128, 8 * NI * 128], BF16, tag="winz")
            nc.sync.dma_start(
                winx[:].rearrange("p (k i n) -> p k i n", k=8, i=NI),
                w_in_x.ap().rearrange("(k p) (i n) -> p k i n", p=128, i=NI))
            nc.sync.dma_start(
                winz[:].rearrange("p (k i n) -> p k i n", k=8, i=NI),
                w_in_z.ap().rearrange("(k p) (i n) -> p k i n", p=128, i=NI))
            wxp = wp.tile([128, NI * (R + 2 * S)], BF16, tag="wxp")
            nc.sync.dma_start(
                wxp[:].rearrange("p (i n) -> p i n", i=NI),
                w_xp.ap().rearrange("(i p) n -> p i n", p=128))
            wdt = wp.tile([R, NI * 128], BF16, tag="wdt")
            nc.sync.dma_start(wdt[:], w_dt[:, :])
            cwd = wp.tile([128, NI * KC * 128], BF16, tag="cwd")
            nc.sync.dma_start(cwd[:], conv_wd[:, :])
            cb = wp.tile([128, NI], F32, tag="cb")
            bdt = wp.tile([128, NI], F32, tag="bdt")
            dv = wp.tile([128, NI], F32, tag="dv")
            am = wp.tile([128, NI * S], F32, tag="am")
            for i in range(NI):
                sl = slice(i * 128, (i + 1) * 128)
                nc.sync.dma_start(cb[:, i:i + 1], conv_b[sl, :])
                nc.sync.dma_start(bdt[:, i:i + 1], b_dt[sl, :])
                nc.sync.dma_start(dv[:, i:i + 1], d_vec[sl, :])
                nc.sync.dma_start(am[:, i * S:(i + 1) * S], a_mat[sl, :])
            bo = wp.tile([128, EO], F32, tag="bo")
            nc.sync.dma_start(bo[:], b_o[:, :])

            hprev = st.tile([128, NI * S], BF16, tag="hprev")
            xtail = st.tile([128, NI * 3], BF16, tag="xtail")

            # A2A input/output DRAM tiles (local, not Shared)
            a2a_in = [dr.tile([2048, tb], BF16, tag=f"a2ain{g}", name=f"a2ain{g}")
                      for g, (_, _, tb) in enumerate(A2A_GROUPS)]
            a2a_out = [dr.tile([2048, tb], BF16, tag=f"a2aout{g}", name=f"a2aout{g}")
                       for g, (_, _, tb) in enumerate(A2A_GROUPS)]

            ctx = {}

            def front(c):
                """x stream, in_proj, conv, silu, z-gate, x_dbl partial, AR issue."""
                t0 = c * TC
                reset = (c % (NCH // B) == 0)

                xck = xkp.tile([128, 8 * TC], BF16, tag="xck")
                nc.sync.dma_start(
                    xck[:].rearrange("p (k t) -> p k t", k=8),
                    x_t.ap()[:, t0:t0 + TC].rearrange("(k p) t -> p k t", p=128))

                xs_i, g_i = [], []
                for i in range(NI):
                    ps_x = psA.tile([128, TC], F32, tag="psx")
                    ps_z = psA.tile([128, TC], F32, tag="psz")
                    for kt in range(8):
                        wsl = slice((kt * NI + i) * 128, (kt * NI + i + 1) * 128)
                        nc.tensor.matmul(ps_x[:], winx[:, wsl], xck[:, kt * TC:(kt + 1) * TC],
                                         start=(kt == 0), stop=(kt == 7))
                    for kt in range(8):
                        wsl = slice((kt * NI + i) * 128, (kt * NI + i + 1) * 128)
                        nc.tensor.matmul(ps_z[:], winz[:, wsl], xck[:, kt * TC:(kt + 1) * TC],
                                         start=(kt == 0), stop=(kt == 7))

                    # causal depthwise conv as 4 accumulating diag matmuls on PE;
                    # xin carries a 3-token halo from the previous chunk
                    xin = wka.tile([128, TC + 3], BF16, tag="xin")
                    nc.scalar.copy(xin[:, 3:], ps_x[:])
                    if reset:
                        nc.gpsimd.memset(xin[:, 0:3], 0.0)
                    else:
                        nc.vector.tensor_copy(xin[:, 0:3], xtail[:, i * 3:i * 3 + 3])
                    nc.vector.tensor_copy(xtail[:, i * 3:i * 3 + 3], ps_x[:, TC - 3:TC])
                    ps_c = psC.tile([128, TC], F32, tag="psc")
                    for k in range(KC):
                        nc.tensor.matmul(ps_c[:], cwd[:, (i * KC + k) * 128:(i * KC + k + 1) * 128],
                                         xin[:, k:k + TC], start=(k == 0), stop=(k == KC - 1))
                    xs = kp.tile([128, TC], BF16, tag="xs")
                    g = kp.tile([128, TC], BF16, tag="g")
                    if use_silu:
                        nc.scalar.activation(xs[:], ps_c[:], AF.Silu, bias=cb[:, i:i + 1])
                        # z gate: g = silu(z) straight from PSUM
                        nc.scalar.activation(g[:], ps_z[:], AF.Silu)
                    else:
                        u = wk.tile([128, TC], F32, tag="accA")
                        nc.scalar.activation(u[:], ps_c[:], AF.Identity, bias=cb[:, i:i + 1])
                        sgu = wka.tile([128, TC], F32, tag="sgu")
                        nc.scalar.activation(sgu[:], u[:], AF.Sigmoid)
                        nc.vector.tensor_tensor(out=xs[:], in0=u[:], in1=sgu[:], op=OP.mult)
                        sgz = wka.tile([128, TC], F32, tag="sgz")
                        nc.scalar.activation(sgz[:], ps_z[:], AF.Sigmoid)
                        nc.vector.scalar_tensor_tensor(
                            out=g[:], in0=sgz[:], scalar=1.0, in1=ps_z[:],
                            op0=OP.mult, op1=OP.mult)
                    xs_i.append(xs)
                    g_i.append(g)

                # x_dbl partial + AllReduce
                ps_xd = psB1.tile([R + 2 * S, TC], F32, tag="psxd")
                for i in range(NI):
                    nc.tensor.matmul(ps_xd[:], wxp[:, i * (R + 2 * S):(i + 1) * (R + 2 * S)],
                                     xs_i[i][:], start=(i == 0), stop=(i == NI - 1))
                xd_sb = wka.tile([R + 2 * S, TC], BF16, tag="xdsb")
                nc.scalar.copy(xd_sb[:], ps_xd[:])
                xd_part = dr.tile([R + 2 * S, TC], BF16, tag=f"xdp{c % 4}")
                nc.sync.dma_start(xd_part[:], xd_sb[:])
                xd_red = nc.dram_tensor(f"xd_red_{c}", [R + 2 * S, TC], BF16, addr_space="Shared")
                nc.gpsimd.collective_compute(
                    "AllReduce", OP.add, replica_groups=[list(range(NC))],
                    ins=[xd_part[:]], outs=[xd_red.ap()])
                ctx[c] = dict(xs_i=xs_i, g_i=g_i, xd_red=xd_red)

            def mid_head(c):
                """dtr/B/C loads and dt for both i-tiles (full chunk width)."""
                xd_red = ctx[c]["xd_red"]
                dtr = wka.tile([R, TC], BF16, tag="dtr")
                nc.sync.dma_start(dtr[:], xd_red.ap()[0:R, :])
                # B/C rows broadcast to all 128 partitions: one stride-0 DMA each
                b_bc = bcp.tile([128, S * TC], BF16, tag="bbc")
                c_bc = bcp.tile([128, S * TC], BF16, tag="cbc")
                nc.scalar.dma_start(
                    b_bc[:].rearrange("p (s t) -> p s t", s=S),
                    xd_red.ap()[R:R + S, :].unsqueeze(0).broadcast_to([128, S, TC]))
                nc.scalar.dma_start(
                    c_bc[:].rearrange("p (s t) -> p s t", s=S),
                    xd_red.ap()[R + S:R + 2 * S, :].unsqueeze(0).broadcast_to([128, S, TC]))
                # Exp+Ln softplus keeps the whole mid phase on the
                # natural_log_exp activation table (a_t uses Exp too)
                dt_i = []
                for i in range(NI):
                    ps_dt = psB2.tile([128, TC], F32, tag="psdt")
                    nc.tensor.matmul(ps_dt[:], wdt[:, i * 128:(i + 1) * 128], dtr[:],
                                     start=True, stop=True)
                    edt = wka.tile([128, TC], BF16, tag="edt")
                    nc.scalar.activation(edt[:], ps_dt[:], AF.Exp, bias=bdt[:, i:i + 1])
                    dt = wka.tile([128, TC], BF16, tag="dtt")
                    nc.scalar.activation(dt[:], edt[:], AF.Ln, bias=1.0)
                    dt_i.append(dt)
                ctx[c].update(b_bc=b_bc, c_bc=c_bc, dt_i=dt_i)

            def mid_body(c, col0, W):
                """dtx, decay exps, bb, scans, h*C + folds, gate, yg scatter for
                token window [c*TC+col0, c*TC+col0+W)."""
                t0 = c * TC + col0
                reset = (t0 % (NT // B) == 0)
                xs_i, g_i = ctx[c]["xs_i"], ctx[c]["g_i"]
                b_bc, c_bc, dt_i = ctx[c]["b_bc"], ctx[c]["c_bc"], ctx[c]["dt_i"]
                win = slice(col0, col0 + W)

                for g, (gt0, gnt, tb) in enumerate(A2A_GROUPS):
                    if gt0 <= t0 < gt0 + gnt:
                        grp, tb_g, rel = g, tb, t0 - gt0
                        break

                # phase A per i-tile: dtx, decay exps, bb, scans, h*C, folds.
                r2_i = []
                for i in range(NI):
                    dt = dt_i[i]
                    dtx = wk.tile([128, TC], BF16, tag="dtx")
                    nc.vector.tensor_tensor(out=dtx[:, 0:W], in0=dt[:, win],
                                            in1=xs_i[i][:, win], op=OP.mult)

                    a_t = sca.tile([128, S * TC], BF16, tag="a_t")
                    for s in range(S):
                        nc.scalar.activation(
                            a_t[:, s * TC + col0:s * TC + col0 + W], dt[:, win], AF.Exp,
                            scale=am[:, i * S + s:i * S + s + 1])

                    bb_t = sc.tile([128, S * TC], BF16, tag="bb_t")
                    bb3 = bb_t[:].rearrange("p (s t) -> p s t", s=S)
                    nc.vector.tensor_tensor(
                        out=bb3[:, :, win],
                        in0=dtx[:, 0:W].unsqueeze(1).broadcast_to([128, S, W]),
                        in1=b_bc[:].rearrange("p (s t) -> p s t", s=S)[:, :, win],
                        op=OP.mult)

                    h_t = sc.tile([128, S * TC], BF16, tag="h_t")
                    if reset:
                        nc.gpsimd.memset(hprev[:, i * S:(i + 1) * S], 0.0)
                    for s in range(S):
                        nc.vector.tensor_tensor_scan(
                            h_t[:, s * TC + col0:s * TC + col0 + W],
                            a_t[:, s * TC + col0:s * TC + col0 + W],
                            bb_t[:, s * TC + col0:s * TC + col0 + W],
                            hprev[:, i * S + s:i * S + s + 1],
                            op0=OP.mult, op1=OP.add)
                    nc.vector.tensor_copy(
                        hprev[:, i * S:(i + 1) * S],
                        h_t[:].rearrange("p (s t) -> p s t", s=S)[:, :, col0 + W - 1])

                    hc_t = sc.tile([128, S * TC], BF16, tag="hc_t", bufs=2)
                    hc3 = hc_t[:].rearrange("p (s t) -> p s t", s=S)
                    nc.vector.tensor_tensor(
                        out=hc3[:, :, win],
                        in0=h_t[:].rearrange("p (s t) -> p s t", s=S)[:, :, win],
                        in1=c_bc[:].rearrange("p (s t) -> p s t", s=S)[:, :, win],
                        op=OP.mult)
                    # folds stay on DVE: serialized but uncontended beats
                    # cross-engine overlap that halves every op's SBUF rate
                    nc.vector.tensor_tensor(out=hc3[:, 0:8, win], in0=hc3[:, 0:8, win],
                                            in1=hc3[:, 8:16, win], op=OP.add)
                    nc.vector.tensor_tensor(out=hc3[:, 0:4, win], in0=hc3[:, 0:4, win],
                                            in1=hc3[:, 4:8, win], op=OP.add)
                    r2_i.append(hc3)

                for i in range(NI):
                    hc3 = r2_i[i]
                    r3 = wk.tile([128, 2 * TC], BF16, tag="r3")
                    nc.vector.tensor_tensor(
                        out=r3[:].rearrange("p (u t) -> p u t", u=2)[:, :, 0:W],
                        in0=hc3[:, 0:2, win], in1=hc3[:, 2:4, win], op=OP.add)
                    y = wk.tile([128, TC], F32, tag="y")
                    nc.vector.tensor_tensor(out=y[:, 0:W], in0=r3[:, 0:W],
                                            in1=r3[:, TC:TC + W], op=OP.add)
                    yD = wk.tile([128, TC], F32, tag="yD")
                    nc.vector.scalar_tensor_tensor(
                        out=yD[:, 0:W], in0=xs_i[i][:, win], scalar=dv[:, i:i + 1],
                        in1=y[:, 0:W], op0=OP.mult, op1=OP.add)
                    yg = wk.tile([128, TC], BF16, tag="yg")
                    nc.vector.tensor_tensor(out=yg[:, 0:W], in0=yD[:, 0:W],
                                            in1=g_i[i][:, win], op=OP.mult)
                    # scatter yg into the A2A input: row-block = token-block,
                    # rows within block = my local d channels
                    for b in range(rel // tb_g, (rel + W + tb_g - 1) // tb_g):
                        colA = max(b * tb_g - rel, 0)
                        colB = min((b + 1) * tb_g - rel, W)
                        nc.sync.dma_start(
                            a2a_in[grp][b * 256 + i * 128: b * 256 + (i + 1) * 128,
                                        rel + colA - b * tb_g: rel + colB - b * tb_g],
                            yg[:, colA:colB])

            def a2a(g):
                nc.gpsimd.collective_compute(
                    "AllToAll", OP.bypass, replica_groups=[list(range(NC))],
                    ins=[a2a_in[g][:]], outs=[a2a_out[g][:]])

            def tail(g):
                """gather a2a_out[g], out projection, bias, store my token cols."""
                tb = A2A_GROUPS[g][2]
                col0 = sum(t for _, _, t in A2A_GROUPS[:g])
                ygs = ygp.tile([128, 16 * tb], BF16, tag="ygs")
                nc.sync.dma_start(
                    ygs[:].rearrange("p (k t) -> p k t", k=16),
                    a2a_out[g][:, :].rearrange("(k p) t -> p k t", p=128))
                for e in range(EO):
                    ps_o = psO.tile([128, tb], F32, tag="pso")
                    for half in range(2):
                        wce = ygp.tile([128, 8 * 128], BF16, tag="wce", bufs=2)
                        nc.sync.dma_start(
                            wce[:], w_c.ap()[:, e * 2048 + half * 1024:e * 2048 + (half + 1) * 1024])
                        for j in range(8):
                            kt = half * 8 + j
                            nc.tensor.matmul(ps_o[:], wce[:, j * 128:(j + 1) * 128],
                                             ygs[:, kt * tb:(kt + 1) * tb],
                                             start=(kt == 0), stop=(kt == 15))
                    o_sb = wka.tile([128, tb], F32, tag="osb")
                    nc.scalar.activation(o_sb[:], ps_o[:], AF.Identity, bias=bo[:, e:e + 1])
                    nc.sync.dma_start(out[e * 128:(e + 1) * 128, col0:col0 + tb], o_sb[:])

            for c in range(NCH + 3):
                if c < NCH:
                    front(c)
                m = c - 2
                if 0 <= m < 7:
                    mid_head(m)
                    mid_body(m, 0, TC)
                    if m == 3:
                        a2a(0)
                    elif m == 5:
                        a2a(1)
                elif m == 7:
                    mid_head(7)
                    mid_body(7, 0, 256)
                    a2a(2)
                    mid_body(7, 256, 256)
                    a2a(3)
                    tail(2)
                if c == 7:
                    tail(0)
                elif c == 8:
                    tail(1)
                elif c == 10:
                    tail(3)

    nc.compile()
    _NC_CACHE[key] = nc
    return nc


def _prep_inputs(inputs):
    x = np.ascontiguousarray(np.asarray(inputs["x"], np.float32))
    W_in = np.asarray(inputs["W_in"], np.float32)
    conv_w = np.asarray(inputs["conv_w"], np.float32)
    conv_b = np.asarray(inputs["conv_b"], np.float32)
    W_xp = np.asarray(inputs["W_xp"], np.float32)
    W_dt = np.asarray(inputs["W_dt"], np.float32)
    b_dt = np.asarray(inputs["b_dt"], np.float32)
    A_log = np.asarray(inputs["A_log"], np.float32)
    D = np.asarray(inputs["D"], np.float32)
    W_out_ssm = np.asarray(inputs["W_out_ssm"], np.float32)
    W_out = np.asarray(inputs["W_out"], np.float32)
    b_out = np.asarray(inputs["b_out"], np.float32)

    A = -np.exp(A_log)
    W_c = (W_out.astype(np.float64) @ W_out_ssm.astype(np.float64)).astype(np.float32)
    x_t = np.ascontiguousarray(x.reshape(NT, DM).T.astype(BFnp))  # [DM, NT] bf16

    # conv taps as diagonal stationary blocks per (local i-tile, tap)
    def conv_diag(dsl):
        cw = conv_w[dsl, 0, :]  # [DIL, KC]
        blk = np.zeros((128, NI * KC * 128), BFnp)
        for i in range(NI):
            for k in range(KC):
                d = np.zeros((128, 128), np.float32)
                np.fill_diagonal(d, cw[i * 128:(i + 1) * 128, k])
                blk[:, (i * KC + k) * 128:(i * KC + k + 1) * 128] = d.astype(BFnp)
        return blk

    # W_c as stationary blocks: [128, (kt*EO+e)*128] = W_c[e-rows, kt-cols].T
    wc_blk = np.zeros((128, 16 * EO * 128), BFnp)
    for e in range(EO):
        for kt in range(16):
            blk = W_c[e * 128:(e + 1) * 128, kt * 128:(kt + 1) * 128].T
            wc_blk[:, (e * 16 + kt) * 128:(e * 16 + kt + 1) * 128] = blk.astype(BFnp)
    bo_blk = np.ascontiguousarray(b_out.reshape(EO, 128).T)  # [128, EO]

    in_maps = []
    for k in range(NC):
        dsl = slice(k * DIL, (k + 1) * DIL)
        in_maps.append({
            "x_t": x_t,
            "w_in_x": np.ascontiguousarray(W_in[dsl, :].T.astype(BFnp)),
            "w_in_z": np.ascontiguousarray(
                W_in[DI + k * DIL: DI + (k + 1) * DIL, :].T.astype(BFnp)),
            "conv_wd": conv_diag(dsl),
            "conv_b": np.ascontiguousarray(conv_b[dsl][:, None]),
            "w_xp": np.ascontiguousarray(W_xp[:, dsl].T.astype(BFnp)),
            "w_dt": np.ascontiguousarray(W_dt[dsl, :].T.astype(BFnp)),
            "b_dt": np.ascontiguousarray(b_dt[dsl][:, None]),
            "a_mat": np.ascontiguousarray(A[dsl, :]),
            "d_vec": np.ascontiguousarray(D[dsl][:, None]),
            "w_c": wc_blk,
            "b_o": bo_blk,
        })
    return in_maps


def _assemble(results):
    full = np.zeros((DM, NT), np.float32)
    for k in range(NC):
        o = results[k]["out"]  # [1024, 512]: token cols per A2A group
        col = 0
        for tok0, ntok, tb in A2A_GROUPS:
            full[:, tok0 + tb * k: tok0 + tb * (k + 1)] = o[:, col:col + tb]
            col += tb
    return np.ascontiguousarray(full.T).reshape(B, L, DM)


def kernel(**inputs):
    nc = build()
    in_maps = _prep_inputs(inputs)
    res = run_bass_kernel_spmd(nc, in_maps, core_ids=list(range(NC)))
    return _assemble(res.results)


def kernel_sim(**inputs):
    """Run through MultiCoreSim instead of HW (for debugging)."""
    from concourse.bass_interp import MultiCoreSim
    nc = build(use_silu=False)
    in_maps = _prep_inputs(inputs)
    sim = MultiCoreSim(nc, num_cores=NC)
    for k in range(NC):
        for name, arr in in_maps[k].items():
            sim.cores[k].tensor(name)[:] = arr
    sim.simulate(check_with_hw=False)
    results = [{"out": sim.cores[k].tensor("out").copy()} for k in range(NC)]
    return _assemble(results)


# revision 42
# speedup vs baseline: 1.0450x; 1.0403x over previous
"""Mamba SSM block on 8 TRN2 NeuronCores (Bass/Tile, SPMD).

Sharding: d_inner (2048 -> 256/core) for in_proj, conv, dt/B/C projections and
the selective scan; output projection is token-sharded (512 tokens/core).
Collectives per run:
  - AllReduce of x_dbl projection partials [96, 512] bf16 per 512-token chunk
  - Three AllToAlls of the gated scan output yg (chunks 0-3 / 4-5 / 6-7) that
    reshard yg from d_inner-sharded to token-sharded; each core then computes
    the full (host-folded) W_c = W_out @ W_out_ssm projection for its tokens.

Scan: h[t] = exp(A dt[t]) h[t-1] + (dt[t] x[t]) B[t] via 16 tensor_tensor_scan
ops (one per state index) along tokens, chained across chunks through
per-partition `initial` APs. B/C are broadcast to 128 partitions by a single
stride-0-partition DMA per chunk (no PE/ACT involvement). silu/softplus run
natively on the ACT engine. Matmul operands and scan tensors are bf16
(rel_err ~5e-3 validated vs fp32 reference); conv, dt and the AllReduce stay
fp32.
"""
import numpy as np
import ml_dtypes

import concourse.bass as bass
import concourse.tile as tile
from concourse import bacc, mybir
from concourse.bass_utils import run_bass_kernel_spmd

BFnp = ml_dtypes.bfloat16
F32 = mybir.dt.float32
BF16 = mybir.dt.bfloat16
AF = mybir.ActivationFunctionType
OP = mybir.AluOpType

NC = 8
B, L, DM = 2, 2048, 1024
DI, S, R, KC = 2048, 16, 64, 4
DIL = DI // NC            # 256 d_inner per core
NT = B * L                # 4096 tokens (batch-major)
TC = 512                  # tokens per chunk
NCH = NT // TC            # 8 chunks
NI = DIL // 128           # 2 partition tiles of local d_inner
EO = 8                    # 1024 output rows = 8 tiles of 128

# AllToAll groups: (token_start, token_count, tokens-per-block)
A2A_GROUPS = [(0, 2048, 256), (2048, 1024, 128), (3072, 512, 64), (3584, 512, 64)]

_NC_CACHE = {}


def _narrow_act_tables(arch):
    """Keep Exp and Ln only in natural_log_exp_and_others so the compiler's
    table-load pass puts the whole dt/a_t sequence on one activation table
    (it otherwise thrashes between exp_and_others and natural_log, ~1.3us
    per reload). Only removes entries, so every emitted load is still valid."""
    from concourse.hw_specs import get_activation_tables
    tabs = get_activation_tables(arch)  # functools.cache -> shared dict
    for name, fns in tabs.items():
        if name == "natural_log_exp_and_others":
            continue
        fns.discard(AF.Exp)
        fns.discard(AF.Ln)


def build(use_silu=True):
    """use_silu=False replaces native Silu with Sigmoid+mult (CoreSim lacks Silu)."""
    key = ("nc", use_silu)
    if key in _NC_CACHE:
        return _NC_CACHE[key]
    nc = bacc.Bacc("TRN2", target_bir_lowering=False, debug=False, num_devices=NC)
    _NC_CACHE[key] = None  # placeholder
    _narrow_act_tables(nc.m.arch)

    # ---- per-core DRAM inputs (host pre-sharded / transposed / casted) ----
    x_t = nc.dram_tensor("x_t", [DM, NT], BF16, kind="ExternalInput")         # replicated
    w_in_x = nc.dram_tensor("w_in_x", [DM, DIL], BF16, kind="ExternalInput")  # W_in[dk,:].T
    w_in_z = nc.dram_tensor("w_in_z", [DM, DIL], BF16, kind="ExternalInput")
    conv_wd = nc.dram_tensor("conv_wd", [128, NI * KC * 128], BF16, kind="ExternalInput")  # diag tap blocks
    conv_b = nc.dram_tensor("conv_b", [DIL, 1], F32, kind="ExternalInput")
    w_xp = nc.dram_tensor("w_xp", [DIL, R + 2 * S], BF16, kind="ExternalInput")  # W_xp[:,dk].T
    w_dt = nc.dram_tensor("w_dt", [R, DIL], BF16, kind="ExternalInput")          # W_dt[dk,:].T
    b_dt = nc.dram_tensor("b_dt", [DIL, 1], F32, kind="ExternalInput")
    a_mat = nc.dram_tensor("a_mat", [DIL, S], F32, kind="ExternalInput")        # -exp(A_log[dk])
    d_vec = nc.dram_tensor("d_vec", [DIL, 1], F32, kind="ExternalInput")
    w_c = nc.dram_tensor("w_c", [128, 16 * EO * 128], BF16, kind="ExternalInput")  # W_c blocks, replicated
    b_o = nc.dram_tensor("b_o", [128, EO], F32, kind="ExternalInput")
    out = nc.dram_tensor("out", [8 * 128, TC], F32, kind="ExternalOutput")     # [1024, my 512 tokens]

    with tile.TileContext(nc) as tc:
        with (
            tc.tile_pool(name="wpool", bufs=1) as wp,     # persistent weights
            tc.tile_pool(name="xckp", bufs=2) as xkp,     # streamed x
            tc.tile_pool(name="ygsp", bufs=1) as ygp,     # A2A output gather
            tc.tile_pool(name="work", bufs=1) as wk,      # DVE-only transients
            tc.tile_pool(name="worka", bufs=2) as wka,    # ACT/DMA-written tiles
            tc.tile_pool(name="keep", bufs=6) as kp,      # xs/g (live across chunk)
            tc.tile_pool(name="scan", bufs=1) as sc,      # big bf16 scan tiles
            tc.tile_pool(name="scana", bufs=2) as sca,    # a_t (dbl buf: ACT vs scan)
            tc.tile_pool(name="bcast", bufs=1) as bcp,    # B/C broadcast tiles
            tc.tile_pool(name="state", bufs=1) as st,     # persistent hprev/xtail
            tc.tile_pool(name="psA", bufs=2, space="PSUM") as psA,
            tc.tile_pool(name="psC", bufs=1, space="PSUM") as psC,
            tc.tile_pool(name="psB1", bufs=1, space="PSUM") as psB1,
            tc.tile_pool(name="psB2", bufs=1, space="PSUM") as psB2,
            tc.tile_pool(name="psO", bufs=1, space="PSUM") as psO,
            tc.tile_pool(name="dram", bufs=1, space="DRAM") as dr,
        ):
            # ---------- load weights ----------
            winx = wp.tile([128, 8 * NI * 128], BF16, tag="winx")
            winz = wp.tile([128, 8 * NI * 128], BF16, tag="winz")
            nc.sync.dma_start(
                winx[:].rearrange("p (k i n) -> p k i n", k=8, i=NI),
                w_in_x.ap().rearrange("(k p) (i n) -> p k i n", p=128, i=NI))
            nc.sync.dma_start(
                winz[:].rearrange("p (k i n) -> p k i n", k=8, i=NI),
                w_in_z.ap().rearrange("(k p) (i n) -> p k i n", p=128, i=NI))
            wxp = wp.tile([128, NI * (R + 2 * S)], BF16, tag="wxp")
            nc.sync.dma_start(
                wxp[:].rearrange("p (i n) -> p i n", i=NI),
                w_xp.ap().rearrange("(i p) n -> p i n", p=128))
            wdt = wp.tile([R, NI * 128], BF16, tag="wdt")
            nc.sync.dma_start(wdt[:], w_dt[:, :])
            cwd = wp.tile([128, NI * KC * 128], BF16, tag="cwd")
            nc.sync.dma_start(cwd[:], conv_wd[:, :])
            cb = wp.tile([128, NI], F32, tag="cb")
            bdt = wp.tile([128, NI], F32, tag="bdt")
            dv = wp.tile([128, NI], F32, tag="dv")
            am = wp.tile([128, NI * S], F32, tag="am")
            for i in range(NI):
                sl = slice(i * 128, (i + 1) * 128)
                nc.sync.dma_start(cb[:, i:i + 1], conv_b[sl, :])
                nc.sync.dma_start(bdt[:, i:i + 1], b_dt[sl, :])
                nc.sync.dma_start(dv[:, i:i + 1], d_vec[sl, :])
                nc.sync.dma_start(am[:, i * S:(i + 1) * S], a_mat[sl, :])
            bo = wp.tile([128, EO], F32, tag="bo")
            nc.sync.dma_start(bo[:], b_o[:, :])

            hprev = st.tile([128, NI * S], BF16, tag="hprev")
            xtail = st.tile([128, NI * 3], BF16, tag="xtail")

            # A2A input/output DRAM tiles (local, not Shared)
            a2a_in = [dr.tile([2048, tb], BF16, tag=f"a2ain{g}", name=f"a2ain{g}")
                      for g, (_, _, tb) in enumerate(A2A_GROUPS)]
            a2a_out = [dr.tile([2048, tb], BF16, tag=f"a2aout{g}", name=f"a2aout{g}")
                       for g, (_, _, tb) in enumerate(A2A_GROUPS)]

            ctx = {}

            def front(c):
                """x stream, in_proj, conv, silu, z-gate, x_dbl partial, AR issue."""
                t0 = c * TC
                reset = (c % (NCH // B) == 0)

                xck = xkp.tile([128, 8 * TC], BF16, tag="xck")
                nc.sync.dma_start(
                    xck[:].rearrange("p (k t) -> p k t", k=8),
                    x_t.ap()[:, t0:t0 + TC].rearrange("(k p) t -> p k t", p=128))

                xs_i, g_i = [], []
                for i in range(NI):
                    ps_x = psA.tile([128, TC], F32, tag="psx")
                    ps_z = psA.tile([128, TC], F32, tag="psz")
                    for kt in range(8):
                        wsl = slice((kt * NI + i) * 128, (kt * NI + i + 1) * 128)
                        nc.tensor.matmul(ps_x[:], winx[:, wsl], xck[:, kt * TC:(kt + 1) * TC],
                                         start=(kt == 0), stop=(kt == 7))
                    for kt in range(8):
                        wsl = slice((kt * NI + i) * 128, (kt * NI + i + 1) * 128)
                        nc.tensor.matmul(ps_z[:], winz[:, wsl], xck[:, kt * TC:(kt + 1) * TC],
                                         start=(kt == 0), stop=(kt == 7))

                    # causal depthwise conv as 4 accumulating diag matmuls on PE;
                    # xin carries a 3-token halo from the previous chunk
                    xin = wka.tile([128, TC + 3], BF16, tag="xin")
                    nc.scalar.copy(xin[:, 3:], ps_x[:])
                    if reset:
                        nc.gpsimd.memset(xin[:, 0:3], 0.0)
                    else:
                        nc.vector.tensor_copy(xin[:, 0:3], xtail[:, i * 3:i * 3 + 3])
                    nc.vector.tensor_copy(xtail[:, i * 3:i * 3 + 3], ps_x[:, TC - 3:TC])
                    ps_c = psC.tile([128, TC], F32, tag="psc")
                    for k in range(KC):
                        nc.tensor.matmul(ps_c[:], cwd[:, (i * KC + k) * 128:(i * KC + k + 1) * 128],
                                         xin[:, k:k + TC], start=(k == 0), stop=(k == KC - 1))
                    xs = kp.tile([128, TC], BF16, tag="xs")
                    g = kp.tile([128, TC], BF16, tag="g")
                    if use_silu:
                        nc.scalar.activation(xs[:], ps_c[:], AF.Silu, bias=cb[:, i:i + 1])
                        # z gate: g = silu(z) straight from PSUM
                        nc.scalar.activation(g[:], ps_z[:], AF.Silu)
                    else:
                        u = wk.tile([128, TC], F32, tag="accA")
                        nc.scalar.activation(u[:], ps_c[:], AF.Identity, bias=cb[:, i:i + 1])
                        sgu = wka.tile([128, TC], F32, tag="sgu")
                        nc.scalar.activation(sgu[:], u[:], AF.Sigmoid)
                        nc.vector.tensor_tensor(out=xs[:], in0=u[:], in1=sgu[:], op=OP.mult)
                        sgz = wka.tile([128, TC], F32, tag="sgz")
                        nc.scalar.activation(sgz[:], ps_z[:], AF.Sigmoid)
                        nc.vector.scalar_tensor_tensor(
                            out=g[:], in0=sgz[:], scalar=1.0, in1=ps_z[:],
                            op0=OP.mult, op1=OP.mult)
                    xs_i.append(xs)
                    g_i.append(g)

                # x_dbl partial + AllReduce
                ps_xd = psB1.tile([R + 2 * S, TC], F32, tag="psxd")
                for i in range(NI):
                    nc.tensor.matmul(ps_xd[:], wxp[:, i * (R + 2 * S):(i + 1) * (R + 2 * S)],
                                     xs_i[i][:], start=(i == 0), stop=(i == NI - 1))
                xd_sb = wka.tile([R + 2 * S, TC], BF16, tag="xdsb")
                nc.scalar.copy(xd_sb[:], ps_xd[:])
                xd_part = dr.tile([R + 2 * S, TC], BF16, tag=f"xdp{c % 4}")
                nc.sync.dma_start(xd_part[:], xd_sb[:])
                xd_red = nc.dram_tensor(f"xd_red_{c}", [R + 2 * S, TC], BF16, addr_space="Shared")
                nc.gpsimd.collective_compute(
                    "AllReduce", OP.add, replica_groups=[list(range(NC))],
                    ins=[xd_part[:]], outs=[xd_red.ap()])
                ctx[c] = dict(xs_i=xs_i, g_i=g_i, xd_red=xd_red)

            def mid_head(c):
                """dtr/B/C loads and dt for both i-tiles (full chunk width)."""
                xd_red = ctx[c]["xd_red"]
                dtr = wka.tile([R, TC], BF16, tag="dtr")
                nc.sync.dma_start(dtr[:], xd_red.ap()[0:R, :])
                # B/C rows broadcast to all 128 partitions: one stride-0 DMA each
                b_bc = bcp.tile([128, S * TC], BF16, tag="bbc")
                c_bc = bcp.tile([128, S * TC], BF16, tag="cbc")
                nc.scalar.dma_start(
                    b_bc[:].rearrange("p (s t) -> p s t", s=S),
                    xd_red.ap()[R:R + S, :].unsqueeze(0).broadcast_to([128, S, TC]))
                nc.scalar.dma_start(
                    c_bc[:].rearrange("p (s t) -> p s t", s=S),
                    xd_red.ap()[R + S:R + 2 * S, :].unsqueeze(0).broadcast_to([128, S, TC]))
                # Exp+Ln softplus keeps the whole mid phase on the
                # natural_log_exp activation table (a_t uses Exp too)
                dt_i = []
                for i in range(NI):
                    ps_dt = psB2.tile([128, TC], F32, tag="psdt")
                    nc.tensor.matmul(ps_dt[:], wdt[:, i * 128:(i + 1) * 128], dtr[:],
                                     start=True, stop=True)
                    edt = wka.tile([128, TC], BF16, tag="edt")
                    nc.scalar.activation(edt[:], ps_dt[:], AF.Exp, bias=bdt[:, i:i + 1])
                    dt = wka.tile([128, TC], BF16, tag="dtt")
                    nc.scalar.activation(dt[:], edt[:], AF.Ln, bias=1.0)
                    dt_i.append(dt)
                ctx[c].update(b_bc=b_bc, c_bc=c_bc, dt_i=dt_i)

            def mid_body(c, col0, W):
                """dtx, decay exps, bb, scans, h*C + folds, gate, yg scatter for
                token window [c*TC+col0, c*TC+col0+W)."""
                t0 = c * TC + col0
                reset = (t0 % (NT // B) == 0)
                xs_i, g_i = ctx[c]["xs_i"], ctx[c]["g_i"]
                b_bc, c_bc, dt_i = ctx[c]["b_bc"], ctx[c]["c_bc"], ctx[c]["dt_i"]
                win = slice(col0, col0 + W)

                for g, (gt0, gnt, tb) in enumerate(A2A_GROUPS):
                    if gt0 <= t0 < gt0 + gnt:
                        grp, tb_g, rel = g, tb, t0 - gt0
                        break

                # phase A per i-tile: dtx, decay exps, bb, scans, h*C, folds.
                r2_i = []
                for i in range(NI):
                    dt = dt_i[i]
                    dtx = wk.tile([128, TC], BF16, tag="dtx")
                    nc.vector.tensor_tensor(out=dtx[:, 0:W], in0=dt[:, win],
                                            in1=xs_i[i][:, win], op=OP.mult)

                    a_t = sca.tile([128, S * TC], BF16, tag="a_t")
                    for s in range(S):
                        nc.scalar.activation(
                            a_t[:, s * TC + col0:s * TC + col0 + W], dt[:, win], AF.Exp,
                            scale=am[:, i * S + s:i * S + s + 1])

                    bb_t = sc.tile([128, S * TC], BF16, tag="bb_t")
                    bb3 = bb_t[:].rearrange("p (s t) -> p s t", s=S)
                    nc.vector.tensor_tensor(
                        out=bb3[:, :, win],
                        in0=dtx[:, 0:W].unsqueeze(1).broadcast_to([128, S, W]),
                        in1=b_bc[:].rearrange("p (s t) -> p s t", s=S)[:, :, win],
                        op=OP.mult)

                    h_t = sc.tile([128, S * TC], BF16, tag="h_t")
                    if reset:
                        nc.gpsimd.memset(hprev[:, i * S:(i + 1) * S], 0.0)
                    for s in range(S):
                        nc.vector.tensor_tensor_scan(
                            h_t[:, s * TC + col0:s * TC + col0 + W],
                            a_t[:, s * TC + col0:s * TC + col0 + W],
                            bb_t[:, s * TC + col0:s * TC + col0 + W],
                            hprev[:, i * S + s:i * S + s + 1],
                            op0=OP.mult, op1=OP.add)
                    nc.vector.tensor_copy(
                        hprev[:, i * S:(i + 1) * S],
                        h_t[:].rearrange("p (s t) -> p s t", s=S)[:, :, col0 + W - 1])

                    hc_t = sc.tile([128, S * TC], BF16, tag="hc_t", bufs=2)
                    hc3 = hc_t[:].rearrange("p (s t) -> p s t", s=S)
                    nc.vector.tensor_tensor(
                        out=hc3[:, :, win],
                        in0=h_t[:].rearrange("p (s t) -> p s t", s=S)[:, :, win],
                        in1=c_bc[:].rearrange("p (s t) -> p s t", s=S)[:, :, win],
                        op=OP.mult)
                    # folds stay on DVE: serialized but uncontended beats
                    # cross-engine overlap that halves every op's SBUF rate
                    nc.vector.tensor_tensor(out=hc3[:, 0:8, win], in0=hc3[:, 0:8, win],
                                            in1=hc3[:, 8:16, win], op=OP.add)
                    nc.vector.tensor_tensor(out=hc3[:, 0:4, win], in0=hc3[:, 0:4, win],
                                            in1=hc3[:, 4:8, win], op=OP.add)
                    r2_i.append(hc3)

                for i in range(NI):
                    hc3 = r2_i[i]
                    r3 = wk.tile([128, 2 * TC], BF16, tag="r3")
                    nc.vector.tensor_tensor(
                        out=r3[:].rearrange("p (u t) -> p u t", u=2)[:, :, 0:W],
                        in0=hc3[:, 0:2, win], in1=hc3[:, 2:4, win], op=OP.add)
                    y = wk.tile([128, TC], F32, tag="y")
                    nc.vector.tensor_tensor(out=y[:, 0:W], in0=r3[:, 0:W],
                                            in1=r3[:, TC:TC + W], op=OP.add)
                    yD = wk.tile([128, TC], F32, tag="yD")
                    nc.vector.scalar_tensor_tensor(
                        out=yD[:, 0:W], in0=xs_i[i][:, win], scalar=dv[:, i:i + 1],
                        in1=y[:, 0:W], op0=OP.mult, op1=OP.add)
                    yg = wk.tile([128, TC], BF16, tag="yg")
                    nc.vector.tensor_tensor(out=yg[:, 0:W], in0=yD[:, 0:W],
                                            in1=g_i[i][:, win], op=OP.mult)
                    # scatter yg into the A2A input: row-block = token-block,
                    # rows within block = my local d channels
                    for b in range(rel // tb_g, (rel + W + tb_g - 1) // tb_g):
                        colA = max(b * tb_g - rel, 0)
                        colB = min((b + 1) * tb_g - rel, W)
                        nc.sync.dma_start(
                            a2a_in[grp][b * 256 + i * 128: b * 256 + (i + 1) * 128,
                                        rel + colA - b * tb_g: rel + colB - b * tb_g],
                            yg[:, colA:colB])

            def a2a(g):
                nc.gpsimd.collective_compute(
                    "AllToAll", OP.bypass, replica_groups=[list(range(NC))],
                    ins=[a2a_in[g][:]], outs=[a2a_out[g][:]])

            def tail(g):
                """gather a2a_out[g], out projection, bias, store my token cols."""
                tb = A2A_GROUPS[g][2]
                col0 = sum(t for _, _, t in A2A_GROUPS[:g])
                ygs = ygp.tile([128, 16 * tb], BF16, tag="ygs")
                nc.sync.dma_start(
                    ygs[:].rearrange("p (k t) -> p k t", k=16),
                    a2a_out[g][:, :].rearrange("(k p) t -> p k t", p=128))
                for e in range(EO):
                    ps_o = psO.tile([128, tb], F32, tag="pso")
                    for half in range(2):
                        wce = ygp.tile([128, 8 * 128], BF16, tag="wce", bufs=2)
                        nc.sync.dma_start(
                            wce[:], w_c.ap()[:, e * 2048 + half * 1024:e * 2048 + (half + 1) * 1024])
                        for j in range(8):
                            kt = half * 8 + j
                            nc.tensor.matmul(ps_o[:], wce[:, j * 128:(j + 1) * 128],
                                             ygs[:, kt * tb:(kt + 1) * tb],
                                             start=(kt == 0), stop=(kt == 15))
                    o_sb = wka.tile([128, tb], F32, tag="osb")
                    nc.scalar.activation(o_sb[:], ps_o[:], AF.Identity, bias=bo[:, e:e + 1])
                    nc.sync.dma_start(out[e * 128:(e + 1) * 128, col0:col0 + tb], o_sb[:])

            for c in range(NCH + 3):
                if c < NCH:
                    front(c)
                m = c - 2
                if 0 <= m < 8:
                    mid_head(m)
                    mid_body(m, 0, TC)
                    if m == 3:
                        a2a(0)
                    elif m == 5:
                        a2a(1)
                    elif m == 6:
                        a2a(2)
                    elif m == 7:
                        a2a(3)
                if c == 7:
                    tail(0)
                elif c == 8:
                    tail(1)
                elif c == 9:
                    tail(2)
                elif c == 10:
                    tail(3)

    nc.compile()
    _NC_CACHE[key] = nc
    return nc


def _prep_inputs(inputs):
    x = np.ascontiguousarray(np.asarray(inputs["x"], np.float32))
    W_in = np.asarray(inputs["W_in"], np.float32)
    conv_w = np.asarray(inputs["conv_w"], np.float32)
    conv_b = np.asarray(inputs["conv_b"], np.float32)
    W_xp = np.asarray(inputs["W_xp"], np.float32)
    W_dt = np.asarray(inputs["W_dt"], np.float32)
    b_dt = np.asarray(inputs["b_dt"], np.float32)
    A_log = np.asarray(inputs["A_log"], np.float32)
    D = np.asarray(inputs["D"], np.float32)
    W_out_ssm = np.asarray(inputs["W_out_ssm"], np.float32)
    W_out = np.asarray(inputs["W_out"], np.float32)
    b_out = np.asarray(inputs["b_out"], np.float32)

    A = -np.exp(A_log)
    W_c = (W_out.astype(np.float64) @ W_out_ssm.astype(np.float64)).astype(np.float32)
    x_t = np.ascontiguousarray(x.reshape(NT, DM).T.astype(BFnp))  # [DM, NT] bf16

    # conv taps as diagonal stationary blocks per (local i-tile, tap)
    def conv_diag(dsl):
        cw = conv_w[dsl, 0, :]  # [DIL, KC]
        blk = np.zeros((128, NI * KC * 128), BFnp)
        for i in range(NI):
            for k in range(KC):
                d = np.zeros((128, 128), np.float32)
                np.fill_diagonal(d, cw[i * 128:(i + 1) * 128, k])
                blk[:, (i * KC + k) * 128:(i * KC + k + 1) * 128] = d.astype(BFnp)
        return blk

    # W_c as stationary blocks: [128, (kt*EO+e)*128] = W_c[e-rows, kt-cols].T
    wc_blk = np.zeros((128, 16 * EO * 128), BFnp)
    for e in range(EO):
        for kt in range(16):
            blk = W_c[e * 128:(e + 1) * 128, kt * 128:(kt + 1) * 128].T
            wc_blk[:, (e * 16 + kt) * 128:(e * 16 + kt + 1) * 128] = blk.astype(BFnp)
    bo_blk = np.ascontiguousarray(b_out.reshape(EO, 128).T)  # [128, EO]

    in_maps = []
    for k in range(NC):
        dsl = slice(k * DIL, (k + 1) * DIL)
        in_maps.append({
            "x_t": x_t,
            "w_in_x": np.ascontiguousarray(W_in[dsl, :].T.astype(BFnp)),
            "w_in_z": np.ascontiguousarray(
                W_in[DI + k * DIL: DI + (k + 1) * DIL, :].T.astype(BFnp)),
            "conv_wd": conv_diag(dsl),
            "conv_b": np.ascontiguousarray(conv_b[dsl][:, None]),
            "w_xp": np.ascontiguousarray(W_xp[:, dsl].T.astype(BFnp)),
            "w_dt": np.ascontiguousarray(W_dt[dsl, :].T.astype(BFnp)),
            "b_dt": np.ascontiguousarray(b_dt[dsl][:, None]),
            "a_mat": np.ascontiguousarray(A[dsl, :]),
            "d_vec": np.ascontiguousarray(D[dsl][:, None]),
            "w_c": wc_blk,
            "b_o": bo_blk,
        })
    return in_maps


def _assemble(results):
    full = np.zeros((DM, NT), np.float32)
    for k in range(NC):
        o = results[k]["out"]  # [1024, 512]: token cols per A2A group
        col = 0
        for tok0, ntok, tb in A2A_GROUPS:
            full[:, tok0 + tb * k: tok0 + tb * (k + 1)] = o[:, col:col + tb]
            col += tb
    return np.ascontiguousarray(full.T).reshape(B, L, DM)


def kernel(**inputs):
    nc = build()
    in_maps = _prep_inputs(inputs)
    res = run_bass_kernel_spmd(nc, in_maps, core_ids=list(range(NC)))
    return _assemble(res.results)


def kernel_sim(**inputs):
    """Run through MultiCoreSim instead of HW (for debugging)."""
    from concourse.bass_interp import MultiCoreSim
    nc = build(use_silu=False)
    in_maps = _prep_inputs(inputs)
    sim = MultiCoreSim(nc, num_cores=NC)
    for k in range(NC):
        for name, arr in in_maps[k].items():
            sim.cores[k].tensor(name)[:] = arr
    sim.simulate(check_with_hw=False)
    results = [{"out": sim.cores[k].tensor("out").copy()} for k in range(NC)]
    return _assemble(results)
